# revision 12
# baseline (speedup 1.0000x reference)
"""DenseCorr2d full kernel for 8 Trainium2 NeuronCores.

Reference computation (per example b):
  corr[(cm*16+ct), y, x] = sum_{dy,dx} tm_edgepad[cm, y+dy, x+dx] * tp[ct, dy, dx]
  out[co, y, x] = bias[co] + sum_{ci,ky,kx} W[co, ci, ky, kx] * corr_zpad[ci, y+ky-1, x+kx-1]

Sharding: data-parallel over batch; core i computes example i entirely.

Stage A (dense correlation) folds the template taps into the matmul
contraction dim: with y = 8w + j and dx = 4a + de, the contraction rows are
(f = j+dy, de) = 92 partitions, the stationary columns are (ct, j) = 128
(fully dense), and accumulation over a happens in PSUM (4 matmuls per
128-col x 512-spatial tile).  The moving operand baseT[cm, (f,de), w, x'] =
tm_pad[cm, 8w+f, x'+de] is precomputed on host so each tile load is a
contiguous-per-partition DMA.  bt loads are chunked per (cm2, ybg) across
the sync+vector queues with two pairs of prefetch so the PE never waits on
HBM; PSUM is allocated per-cm (4 banks + 4 spare) so evacuation of cm N
overlaps the matmuls of cm N+1 with no bank-reuse stall.

corr lives in SBUF j-interleaved: corr[ci, (c, j, ws, x)] holds row
y = 8*(ws-1) + j of chunk c (ws=0/17 are the y=-1/128 zero-pad rows, x is
130 wide with zero x-borders).  The PSUM evacuation is a full-rate copy
(cast fp32->bf16 into a staging tile) plus two half-slab DMAs per cm
(split by ct so both run in parallel on different queues).

Stage B runs the 3x3 'same' merge conv over residue bands as ROW-PAIRED
matmuls: one [128, 2, 64] stationary holds the weights for two adjacent
output rows (j0, j0+1) that share the same corr slab row jj = j0+dd-1
(slot s covers row j0+s with ky = dd-s; invalid (dd,s) combos hold zero
weights).  That cuts the matmuls per block from 18 to 12 per output row
and uses the full 128 PE columns on the interior taps.  Each slot
evacuates to its own partition range of the output tile (scalar adds the
bias), and the output DMA un-interleaves the (slot, co) partitions into
the right y rows.

All matmuls run in bf16 (inputs are unit-normal; accumulation in fp32
PSUM keeps the relative error ~4e-3, well inside the 2e-2 gate).
"""

from contextlib import ExitStack

import ml_dtypes
import numpy as np

import concourse.bass as bass
import concourse.tile as tile
from concourse import bacc, mybir
from concourse.bass_utils import run_bass_kernel_spmd

F32 = mybir.dt.float32
BF16 = mybir.dt.bfloat16

N_CORES = 8
# Problem shapes (hardcoded per contract).
B, CT, HT, WT = 8, 16, 16, 16
CM, HM, WM = 16, 128, 128
COUT, K = 64, 3
HP = HM + HT - 1  # 143 padded image rows/cols
NF = 23  # f = j + dy range
NP = 4 * NF  # 92 contraction rows (f, de); de in [0,4), dx = 4a + de
NA = 4  # PSUM accumulation steps over a
XW = 140  # x' range of baseT (x + 4a <= 127+12)
SLAB = 18 * 130  # corr slab (c, j): 18 ws-rows of 130
NCORR = 16 * SLAB  # 2 chunks * 8 j

# stage-B w-window split: 16 w values per j0-residue, psum <= 512 fp32
RWS = [3, 3, 3, 3, 2, 2]
W0S = [0, 3, 6, 9, 12, 14]

_CACHE: dict = {}


def _emit(ctx: ExitStack, tc, nc, btT, sa2, wst, bia, zzb, out):
    const = ctx.enter_context(tc.tile_pool(name="const", bufs=1))
    corrp = ctx.enter_context(tc.tile_pool(name="corrp", bufs=1))

    # sa2 rides the scalar queue so the sync queue's first op is the first
    # bt chunk; both are needed by matmul #1.  w goes to the otherwise-idle
    # gpsimd SWDGE so it never delays loads or shuffles.
    sa2_sb = const.tile([NP, NA, 128], BF16, name="sa2_sb")
    nc.scalar.dma_start(out=sa2_sb[:], in_=sa2.ap())
    b_sb = const.tile([128, 1], F32, name="b_sb")
    nc.scalar.dma_start(out=b_sb[:], in_=bia.ap())
    # row-paired stage-B weights: [k, (c, dd, kx), s, co]
    w_sb = const.tile([128, 24, 2, COUT], BF16, name="w_sb")
    nc.gpsimd.dma_start(out=w_sb[:, :12], in_=wst.ap()[:, :12])
    nc.gpsimd.dma_start(out=w_sb[:, 12:], in_=wst.ap()[:, 12:])

    corr_sb = corrp.tile([128, NCORR + 2], BF16, name="corr_sb")
    corr_flat = corr_sb[:]
    # slab view: [p, c*8+j, ws*130+x]
    corr_j = corr_sb[:, :NCORR].rearrange("p (s t) -> p s t", s=16)
    # tail slack (zero-pad rows/cols arrive with the per-cm slab shuffle)
    nc.scalar.dma_start(out=corr_sb[:, NCORR:], in_=zzb.ap()[:, :2])

    # ---- Stage A ----
    with (
        tc.tile_pool(name="btp", bufs=3) as btp,
        tc.tile_pool(name="stp0", bufs=2) as stp0,
        tc.tile_pool(name="stp1", bufs=2) as stp1,
        tc.tile_pool(name="psA", bufs=8, space="PSUM") as psA,
    ):
        stps = [stp0, stp1]

        def load_pair(pr):
            # one big DMA per cm2-half (few descriptors, long runs — the HW
            # splits a big InstDMACopy across SDMA engines at packet
            # granularity), cm2=0 on sync and cm2=1 on scalar so the two
            # halves pull in parallel.  Pair 0's sync half is split once
            # more so the first matmul starts after ~200KB.
            t = btp.tile([NP, 2, 16, XW], BF16, name="bt", tag="bt")
            if pr == 0:
                nc.sync.dma_start(
                    out=t[:, 0, 0:8], in_=btT.ap()[0, :, 0, 0:8]
                )
                nc.sync.dma_start(
                    out=t[:, 0, 8:16], in_=btT.ap()[0, :, 0, 8:16]
                )
            else:
                nc.sync.dma_start(out=t[:, 0], in_=btT.ap()[pr, :, 0])
            nc.sync.dma_start(out=t[:, 1], in_=btT.ap()[pr, :, 1])
            return t

        loads = {0: load_pair(0), 1: load_pair(1)}
        for pr in range(8):
            bt = loads.pop(pr)
            if pr + 2 < 8:
                loads[pr + 2] = load_pair(pr + 2)
            for cm2 in range(2):
                cm = 2 * pr + cm2
                pts = [
                    psA.tile([128, 4, 128], F32, name=f"pA{ybg}", tag="pA")
                    for ybg in range(4)
                ]
                # a-inner so each ybg-block only needs its own bt chunk; the
                # 4 accumulating matmuls to one bank run back-to-back.
                for ybg in range(4):
                    for a in range(NA):
                        nc.tensor.matmul(
                            pts[ybg][:],
                            sa2_sb[:, a, :],
                            bt[:, cm2, 4 * ybg : 4 * ybg + 4, 4 * a : 4 * a + 128],
                            start=(a == 0),
                            stop=(a == NA - 1),
                        )
                # st covers the full 18-row slab: ws=0/17 zero-pad rows and
                # x-border zeros ride along with the data in one shuffle.
                st = stps[cm2].tile([128, 18, 130], BF16, name="st", tag="st")
                nc.vector.memset(st[:, 0, :], 0.0)
                nc.vector.memset(st[:, 17, :], 0.0)
                nc.vector.memset(st[:, 1:17, 0], 0.0)
                nc.vector.memset(st[:, 1:17, 129], 0.0)
                for ybg in range(4):
                    dst_sl = st[:, 1 + 4 * ybg : 5 + 4 * ybg, 1:129]
                    if ybg % 2 == 0:
                        nc.vector.tensor_copy(dst_sl, pts[ybg][:])
                    else:
                        nc.scalar.copy(dst_sl, pts[ybg][:])
                # st[(8ct+j), ws, x] -> corr slabs (c, j), split by ct-half.
                # Shuffles ride gpsimd SWDGE rings (DMA engines 4-15, which
                # sit idle) so the 4 engines behind the HWDGE queues carry
                # only bt loads; the last cms use the by-then-idle HWDGE
                # queues so the SWDGE path is never the stage-A tail.
                c, cmh = cm // 8, cm % 8
                if cm >= 14:
                    # quarters across all queues so the tail shuffles land
                    # fast (loads are done; HWDGE engines are idle by now)
                    for qi, eng in enumerate(
                        (nc.sync, nc.scalar, nc.sync, nc.scalar)
                    ):
                        eng.dma_start(
                            out=corr_j[
                                16 * cmh + 4 * qi : 16 * cmh + 4 * qi + 4,
                                c * 8 : c * 8 + 8,
                                :,
                            ],
                            in_=st[32 * qi : 32 * qi + 32],
                        )
                else:
                    nc.gpsimd.dma_start(
                        out=corr_j[16 * cmh : 16 * cmh + 8, c * 8 : c * 8 + 8, :],
                        in_=st[0:64],
                    )
                    nc.gpsimd.dma_start(
                        out=corr_j[
                            16 * cmh + 8 : 16 * cmh + 16, c * 8 : c * 8 + 8, :
                        ],
                        in_=st[64:128],
                    )

    # ---- Stage B ----
    def slab_off(c, jj, w0):
        if jj < 0:
            return (c * 8 + 7) * SLAB + w0 * 130
        if jj > 7:
            return (c * 8 + jj - 8) * SLAB + (w0 + 2) * 130
        return (c * 8 + jj) * SLAB + (w0 + 1) * 130

    # out dram view for the un-interleave: [s, co, j0p, w, x]
    out_v = out.ap().rearrange("co (w jp s) x -> s co jp w x", jp=4, s=2)

    with (
        tc.tile_pool(name="psB", bufs=6, space="PSUM") as psB,
        tc.tile_pool(name="outp", bufs=3) as outp,
    ):
        # software-pipelined: emit chunk-0 taps of block k, then chunk-1 taps
        # of block k-1, so the PE always has chunk-0 work (ready early) while
        # the last chunk-1 shuffles land.
        blocks = [(wg, j0p) for wg in range(len(RWS)) for j0p in range(4)]
        state = {}  # (wg,j0p) -> pb

        def emit_chunk(key, c):
            wg, j0p = key
            w0, rw = W0S[wg], RWS[wg]
            n = rw * 130
            if c == 0:
                pb = psB.tile([128, n], F32, name="pb", tag="pb")
                state[key] = pb
            else:
                pb = state.pop(key)
            for dd in range(4):
                off0 = slab_off(c, 2 * j0p + dd - 1, w0)
                for kx in range(3):
                    nc.tensor.matmul(
                        pb[:],
                        w_sb[:, c * 12 + dd * 3 + kx, :, :],
                        corr_flat[:, off0 + kx : off0 + kx + n],
                        start=(c == 0 and dd == 0 and kx == 0),
                        stop=(c == 1 and dd == 3 and kx == 2),
                    )
            if c == 1:
                # bias + evacuate both row-slots at once; partitions are
                # (s, co), un-interleaved by the per-slot output DMAs.
                ot = outp.tile([128, rw, WM], F32, name="ot", tag="ot")
                nc.scalar.activation(
                    ot[:],
                    pb[:].rearrange("p (a b) -> p a b", b=130)[:, :, 0:128],
                    mybir.ActivationFunctionType.Identity,
                    bias=b_sb[:, 0:1],
                )
                for s in range(2):
                    nc.sync.dma_start(
                        out=out_v[s, :, j0p, w0 : w0 + rw, :],
                        in_=ot[64 * s : 64 * s + 64],
                    )

        # two-deep software pipeline: c0(b0) c0(b1) c0(b2) c1(b0) c0(b3)
        # c1(b1) ... so ~8us of chunk-0 work (ready early) bridges the wait
        # for the last chunk-1 shuffles.
        DEPTH = 2
        pend = []
        for key in blocks:
            emit_chunk(key, 0)
            pend.append(key)
            if len(pend) > DEPTH:
                emit_chunk(pend.pop(0), 1)
        for key in pend:
            emit_chunk(key, 1)


def _build():
    nc = bacc.Bacc("TRN2", target_bir_lowering=False, debug=False)
    btT = nc.dram_tensor("btT", [8, NP, 2, 16, XW], BF16, kind="ExternalInput")
    sa2 = nc.dram_tensor("sa2", [NP, NA, 128], BF16, kind="ExternalInput")
    wst = nc.dram_tensor("wst", [128, 24, 2, COUT], BF16, kind="ExternalInput")
    bia = nc.dram_tensor("bias", [128, 1], F32, kind="ExternalInput")
    zzb = nc.dram_tensor("zzb", [128, 130], BF16, kind="ExternalInput")
    out = nc.dram_tensor("out", [COUT, HM, WM], F32, kind="ExternalOutput")
    with tile.TileContext(nc) as tc, ExitStack() as ctx:
        _emit(ctx, tc, nc, btT, sa2, wst, bia, zzb, out)
    nc.compile()
    return nc


def _get_nc():
    if "nc" not in _CACHE:
        _CACHE["nc"] = _build()
    return _CACHE["nc"]


def _host_prep(template, tomatch, W, b):
    template = np.ascontiguousarray(template, dtype=np.float32)
    tomatch = np.ascontiguousarray(tomatch, dtype=np.float32)
    W = np.ascontiguousarray(W, dtype=np.float32)
    b = np.ascontiguousarray(b, dtype=np.float32)
    bf = ml_dtypes.bfloat16

    tm_pad = np.pad(
        tomatch, ((0, 0), (0, 0), (0, HT - 1), (0, WT - 1)), mode="edge"
    )  # [B, CM, 143, 143]

    # baseT[b, cm, 4f+de, w, x'] = tm_pad[b, cm, 8w+f, x'+de]
    s0, s1, s2, s3 = tm_pad.strides
    bview = np.lib.stride_tricks.as_strided(
        tm_pad,
        shape=(B, CM, NF, 4, 16, XW),
        strides=(s0, s1, s2, s3, 8 * s2, s3),
    )
    # pair-grouped loads: [b, pr, p, cm2, w, x']
    btT = (
        np.ascontiguousarray(bview)
        .reshape(B, 8, 2, NP, 16, XW)
        .transpose(0, 1, 3, 2, 4, 5)
        .astype(bf)
    )
    btT = np.ascontiguousarray(btT)

    # sa2[b, 4f+de, a, 8ct+j] = template[b, ct, f-j, 4a+de] for 0<=f-j<16
    sa2 = np.zeros((B, NP, NA, 128), np.float32)
    tview = template.reshape(B, CT, HT, NA, 4)  # [b, ct, dy, a, de]
    for j in range(8):
        for dy in range(HT):
            f = j + dy
            # [b, de, a, ct] slab
            sa2[:, 4 * f : 4 * f + 4, :, j::8] = tview[:, :, dy].transpose(
                0, 3, 2, 1
            )
    sa2 = sa2.astype(bf)

    # row-paired merge-conv weights:
    #   wst[k, (c, dd, kx), s, co] = W[co, c*128+k, dd-s, kx]  (0 <= dd-s < 3)
    # slot s covers output row j0+s; dd = jj - (j0-1) indexes the corr slab
    # row the pair shares.
    Wr = W.reshape(COUT, 2, 128, K, K)  # [co, c, k, ky, kx]
    wst = np.zeros((128, 2, 4, K, 2, COUT), np.float32)  # [k, c, dd, kx, s, co]
    for dd in range(4):
        for s in range(2):
            ky = dd - s
            if 0 <= ky < K:
                # [k, c, kx, co]
                wst[:, :, dd, :, s, :] = Wr[:, :, :, ky, :].transpose(2, 1, 3, 0)
    wst = np.ascontiguousarray(wst.reshape(128, 24, 2, COUT)).astype(bf)

    bias = np.ascontiguousarray(np.tile(b, 2).reshape(128, 1))
    zzb = np.zeros((128, 130), bf)
    return btT, sa2, wst, bias, zzb


def _in_maps(template, tomatch, W, b):
    btT, sa2, wst, bias, zzb = _host_prep(template, tomatch, W, b)
    return [
        {"btT": btT[i], "sa2": sa2[i], "wst": wst, "bias": bias, "zzb": zzb}
        for i in range(N_CORES)
    ]


def kernel(template, tomatch, W, b):
    in_maps = _in_maps(template, tomatch, W, b)
    nc = _get_nc()
    res = run_bass_kernel_spmd(nc, in_maps, list(range(N_CORES)))
    return np.stack([res.results[i]["out"] for i in range(N_CORES)])


# revision 14
# speedup vs baseline: 1.0623x; 1.0623x over previous
"""DenseCorr2d full kernel for 8 Trainium2 NeuronCores.

Reference computation (per example b):
  corr[(cm*16+ct), y, x] = sum_{dy,dx} tm_edgepad[cm, y+dy, x+dx] * tp[ct, dy, dx]
  out[co, y, x] = bias[co] + sum_{ci,ky,kx} W[co, ci, ky, kx] * corr_zpad[ci, y+ky-1, x+kx-1]

Sharding: data-parallel over batch; core i computes example i entirely.

Stage A (dense correlation) folds the template taps into the matmul
contraction dim: with y = 8w + j and dx = 4a + de, the contraction rows are
(f = j+dy, de) = 92 partitions, the stationary columns are (ct, j) = 128
(fully dense), and accumulation over a happens in PSUM (4 matmuls per
128-col x 512-spatial tile).  The moving operand baseT[cm, (f,de), w, x'] =
tm_pad[cm, 8w+f, x'+de] is precomputed on host so each tile load is a
contiguous-per-partition DMA.  bt loads are chunked per (cm2, ybg) across
the sync+vector queues with two pairs of prefetch so the PE never waits on
HBM; PSUM is allocated per-cm (4 banks + 4 spare) so evacuation of cm N
overlaps the matmuls of cm N+1 with no bank-reuse stall.

corr lives in SBUF j-interleaved: corr[ci, (c, j, ws, x)] holds row
y = 8*(ws-1) + j of chunk c (ws=0/17 are the y=-1/128 zero-pad rows, x is
130 wide with zero x-borders).  The PSUM evacuation is a full-rate copy
(cast fp32->bf16 into a staging tile) plus two half-slab DMAs per cm
(split by ct so both run in parallel on different queues).

Stage B runs the 3x3 'same' merge conv over residue bands as ROW-PAIRED
matmuls: one [128, 2, 64] stationary holds the weights for two adjacent
output rows (j0, j0+1) that share the same corr slab row jj = j0+dd-1
(slot s covers row j0+s with ky = dd-s; invalid (dd,s) combos hold zero
weights).  That cuts the matmuls per block from 18 to 12 per output row
and uses the full 128 PE columns on the interior taps.  Each slot
evacuates to its own partition range of the output tile (scalar adds the
bias), and the output DMA un-interleaves the (slot, co) partitions into
the right y rows.

All matmuls run in bf16 (inputs are unit-normal; accumulation in fp32
PSUM keeps the relative error ~4e-3, well inside the 2e-2 gate).
"""

from contextlib import ExitStack

import ml_dtypes
import numpy as np

import concourse.bass as bass
import concourse.tile as tile
from concourse import bacc, mybir
from concourse.bass_utils import run_bass_kernel_spmd

F32 = mybir.dt.float32
BF16 = mybir.dt.bfloat16

N_CORES = 8
# Problem shapes (hardcoded per contract).
B, CT, HT, WT = 8, 16, 16, 16
CM, HM, WM = 16, 128, 128
COUT, K = 64, 3
HP = HM + HT - 1  # 143 padded image rows/cols
NF = 23  # f = j + dy range
NP = 4 * NF  # 92 contraction rows (f, de); de in [0,4), dx = 4a + de
NA = 4  # PSUM accumulation steps over a
XW = 140  # x' range of baseT (x + 4a <= 127+12)
SLAB = 18 * 130  # corr slab (c, j): 18 ws-rows of 130
NCORR = 16 * SLAB  # 2 chunks * 8 j

# stage-B w-window split: 16 w values per j0-residue, psum <= 512 fp32
RWS = [3, 3, 3, 3, 2, 2]
W0S = [0, 3, 6, 9, 12, 14]

_CACHE: dict = {}


def _emit(ctx: ExitStack, tc, nc, btT, sa2, wst, bia, zzb, out):
    const = ctx.enter_context(tc.tile_pool(name="const", bufs=1))
    corrp = ctx.enter_context(tc.tile_pool(name="corrp", bufs=1))

    # sa2 rides the scalar queue so the sync queue's first op is the first
    # bt chunk; both are needed by matmul #1.  w goes to the otherwise-idle
    # gpsimd SWDGE so it never delays loads or shuffles.
    sa2_sb = const.tile([NP, NA, 128], BF16, name="sa2_sb")
    nc.scalar.dma_start(out=sa2_sb[:], in_=sa2.ap())
    b_sb = const.tile([128, 1], F32, name="b_sb")
    nc.scalar.dma_start(out=b_sb[:], in_=bia.ap())
    # row-paired stage-B weights: [k, (c, dd, kx), s, co]
    w_sb = const.tile([128, 24, 2, COUT], BF16, name="w_sb")
    nc.gpsimd.dma_start(out=w_sb[:, :12], in_=wst.ap()[:, :12])
    nc.gpsimd.dma_start(out=w_sb[:, 12:], in_=wst.ap()[:, 12:])

    corr_sb = corrp.tile([128, NCORR + 2], BF16, name="corr_sb")
    corr_flat = corr_sb[:]
    # slab view: [p, c*8+j, ws*130+x]
    corr_j = corr_sb[:, :NCORR].rearrange("p (s t) -> p s t", s=16)
    # tail slack (zero-pad rows/cols arrive with the per-cm slab shuffle)
    nc.scalar.dma_start(out=corr_sb[:, NCORR:], in_=zzb.ap()[:, :2])

    # ---- Stage A ----
    with (
        tc.tile_pool(name="btp", bufs=4) as btp,
        tc.tile_pool(name="stp0", bufs=2) as stp0,
        tc.tile_pool(name="stp1", bufs=2) as stp1,
        tc.tile_pool(name="psA", bufs=8, space="PSUM") as psA,
    ):
        stps = [stp0, stp1]

        def load_pair(pr):
            # one big DMA per cm2-half (few descriptors, long runs — the HW
            # splits a big InstDMACopy across SDMA engines at packet
            # granularity), cm2=0 on sync and cm2=1 on scalar so the two
            # halves pull in parallel.  Pair 0's sync half is split once
            # more so the first matmul starts after ~200KB.
            t = btp.tile([NP, 2, 16, XW], BF16, name="bt", tag="bt")
            if pr == 0:
                nc.sync.dma_start(
                    out=t[:, 0, 0:8], in_=btT.ap()[0, :, 0, 0:8]
                )
                nc.sync.dma_start(
                    out=t[:, 0, 8:16], in_=btT.ap()[0, :, 0, 8:16]
                )
            else:
                nc.sync.dma_start(out=t[:, 0], in_=btT.ap()[pr, :, 0])
            nc.scalar.dma_start(out=t[:, 1], in_=btT.ap()[pr, :, 1])
            return t

        loads = {pr: load_pair(pr) for pr in range(3)}
        for pr in range(8):
            bt = loads.pop(pr)
            if pr + 3 < 8:
                loads[pr + 3] = load_pair(pr + 3)
            for cm2 in range(2):
                cm = 2 * pr + cm2
                pts = [
                    psA.tile([128, 4, 128], F32, name=f"pA{ybg}", tag="pA")
                    for ybg in range(4)
                ]
                # a-inner so each ybg-block only needs its own bt chunk; the
                # 4 accumulating matmuls to one bank run back-to-back.
                for ybg in range(4):
                    for a in range(NA):
                        nc.tensor.matmul(
                            pts[ybg][:],
                            sa2_sb[:, a, :],
                            bt[:, cm2, 4 * ybg : 4 * ybg + 4, 4 * a : 4 * a + 128],
                            start=(a == 0),
                            stop=(a == NA - 1),
                        )
                # st covers the full 18-row slab: ws=0/17 zero-pad rows and
                # x-border zeros ride along with the data in one shuffle.
                st = stps[cm2].tile([128, 18, 130], BF16, name="st", tag="st")
                nc.vector.memset(st[:, 0, :], 0.0)
                nc.vector.memset(st[:, 17, :], 0.0)
                nc.vector.memset(st[:, 1:17, 0], 0.0)
                nc.vector.memset(st[:, 1:17, 129], 0.0)
                for ybg in range(4):
                    dst_sl = st[:, 1 + 4 * ybg : 5 + 4 * ybg, 1:129]
                    if ybg % 2 == 0:
                        nc.vector.tensor_copy(dst_sl, pts[ybg][:])
                    else:
                        nc.scalar.copy(dst_sl, pts[ybg][:])
                # st[(8ct+j), ws, x] -> corr slabs (c, j), split by ct-half.
                # Shuffles ride gpsimd SWDGE rings (DMA engines 4-15, which
                # sit idle) so the 4 engines behind the HWDGE queues carry
                # only bt loads; the last cms use the by-then-idle HWDGE
                # queues so the SWDGE path is never the stage-A tail.
                c, cmh = cm // 8, cm % 8
                if cm >= 14:
                    # quarters across all queues so the tail shuffles land
                    # fast (loads are done; HWDGE engines are idle by now)
                    for qi, eng in enumerate(
                        (nc.sync, nc.scalar, nc.sync, nc.scalar)
                    ):
                        eng.dma_start(
                            out=corr_j[
                                16 * cmh + 4 * qi : 16 * cmh + 4 * qi + 4,
                                c * 8 : c * 8 + 8,
                                :,
                            ],
                            in_=st[32 * qi : 32 * qi + 32],
                        )
                else:
                    nc.gpsimd.dma_start(
                        out=corr_j[16 * cmh : 16 * cmh + 8, c * 8 : c * 8 + 8, :],
                        in_=st[0:64],
                    )
                    nc.gpsimd.dma_start(
                        out=corr_j[
                            16 * cmh + 8 : 16 * cmh + 16, c * 8 : c * 8 + 8, :
                        ],
                        in_=st[64:128],
                    )

    # ---- Stage B ----
    def slab_off(c, jj, w0):
        if jj < 0:
            return (c * 8 + 7) * SLAB + w0 * 130
        if jj > 7:
            return (c * 8 + jj - 8) * SLAB + (w0 + 2) * 130
        return (c * 8 + jj) * SLAB + (w0 + 1) * 130

    # out dram view for the un-interleave: [s, co, j0p, w, x]
    out_v = out.ap().rearrange("co (w jp s) x -> s co jp w x", jp=4, s=2)

    with (
        tc.tile_pool(name="psB", bufs=6, space="PSUM") as psB,
        tc.tile_pool(name="outp", bufs=3) as outp,
    ):
        # software-pipelined: emit chunk-0 taps of block k, then chunk-1 taps
        # of block k-1, so the PE always has chunk-0 work (ready early) while
        # the last chunk-1 shuffles land.
        blocks = [(wg, j0p) for wg in range(len(RWS)) for j0p in range(4)]
        state = {}  # (wg,j0p) -> pb

        def emit_chunk(key, c):
            wg, j0p = key
            w0, rw = W0S[wg], RWS[wg]
            n = rw * 130
            if c == 0:
                pb = psB.tile([128, n], F32, name="pb", tag="pb")
                state[key] = pb
            else:
                pb = state.pop(key)
            for dd in range(4):
                off0 = slab_off(c, 2 * j0p + dd - 1, w0)
                for kx in range(3):
                    nc.tensor.matmul(
                        pb[:],
                        w_sb[:, c * 12 + dd * 3 + kx, :, :],
                        corr_flat[:, off0 + kx : off0 + kx + n],
                        start=(c == 0 and dd == 0 and kx == 0),
                        stop=(c == 1 and dd == 3 and kx == 2),
                    )
            if c == 1:
                # bias + evacuate both row-slots at once; partitions are
                # (s, co), un-interleaved by the per-slot output DMAs.
                ot = outp.tile([128, rw, WM], F32, name="ot", tag="ot")
                nc.scalar.activation(
                    ot[:],
                    pb[:].rearrange("p (a b) -> p a b", b=130)[:, :, 0:128],
                    mybir.ActivationFunctionType.Identity,
                    bias=b_sb[:, 0:1],
                )
                for s in range(2):
                    nc.sync.dma_start(
                        out=out_v[s, :, j0p, w0 : w0 + rw, :],
                        in_=ot[64 * s : 64 * s + 64],
                    )

        # two-deep software pipeline: c0(b0) c0(b1) c0(b2) c1(b0) c0(b3)
        # c1(b1) ... so ~8us of chunk-0 work (ready early) bridges the wait
        # for the last chunk-1 shuffles.
        DEPTH = 3
        pend = []
        for key in blocks:
            emit_chunk(key, 0)
            pend.append(key)
            if len(pend) > DEPTH:
                emit_chunk(pend.pop(0), 1)
        for key in pend:
            emit_chunk(key, 1)


def _build():
    nc = bacc.Bacc("TRN2", target_bir_lowering=False, debug=False)
    btT = nc.dram_tensor("btT", [8, NP, 2, 16, XW], BF16, kind="ExternalInput")
    sa2 = nc.dram_tensor("sa2", [NP, NA, 128], BF16, kind="ExternalInput")
    wst = nc.dram_tensor("wst", [128, 24, 2, COUT], BF16, kind="ExternalInput")
    bia = nc.dram_tensor("bias", [128, 1], F32, kind="ExternalInput")
    zzb = nc.dram_tensor("zzb", [128, 130], BF16, kind="ExternalInput")
    out = nc.dram_tensor("out", [COUT, HM, WM], F32, kind="ExternalOutput")
    with tile.TileContext(nc) as tc, ExitStack() as ctx:
        _emit(ctx, tc, nc, btT, sa2, wst, bia, zzb, out)
    nc.compile()
    return nc


def _get_nc():
    if "nc" not in _CACHE:
        _CACHE["nc"] = _build()
    return _CACHE["nc"]


def _host_prep(template, tomatch, W, b):
    template = np.ascontiguousarray(template, dtype=np.float32)
    tomatch = np.ascontiguousarray(tomatch, dtype=np.float32)
    W = np.ascontiguousarray(W, dtype=np.float32)
    b = np.ascontiguousarray(b, dtype=np.float32)
    bf = ml_dtypes.bfloat16

    tm_pad = np.pad(
        tomatch, ((0, 0), (0, 0), (0, HT - 1), (0, WT - 1)), mode="edge"
    )  # [B, CM, 143, 143]

    # baseT[b, cm, 4f+de, w, x'] = tm_pad[b, cm, 8w+f, x'+de]
    s0, s1, s2, s3 = tm_pad.strides
    bview = np.lib.stride_tricks.as_strided(
        tm_pad,
        shape=(B, CM, NF, 4, 16, XW),
        strides=(s0, s1, s2, s3, 8 * s2, s3),
    )
    # pair-grouped loads: [b, pr, p, cm2, w, x']
    btT = (
        np.ascontiguousarray(bview)
        .reshape(B, 8, 2, NP, 16, XW)
        .transpose(0, 1, 3, 2, 4, 5)
        .astype(bf)
    )
    btT = np.ascontiguousarray(btT)

    # sa2[b, 4f+de, a, 8ct+j] = template[b, ct, f-j, 4a+de] for 0<=f-j<16
    sa2 = np.zeros((B, NP, NA, 128), np.float32)
    tview = template.reshape(B, CT, HT, NA, 4)  # [b, ct, dy, a, de]
    for j in range(8):
        for dy in range(HT):
            f = j + dy
            # [b, de, a, ct] slab
            sa2[:, 4 * f : 4 * f + 4, :, j::8] = tview[:, :, dy].transpose(
                0, 3, 2, 1
            )
    sa2 = sa2.astype(bf)

    # row-paired merge-conv weights:
    #   wst[k, (c, dd, kx), s, co] = W[co, c*128+k, dd-s, kx]  (0 <= dd-s < 3)
    # slot s covers output row j0+s; dd = jj - (j0-1) indexes the corr slab
    # row the pair shares.
    Wr = W.reshape(COUT, 2, 128, K, K)  # [co, c, k, ky, kx]
    wst = np.zeros((128, 2, 4, K, 2, COUT), np.float32)  # [k, c, dd, kx, s, co]
    for dd in range(4):
        for s in range(2):
            ky = dd - s
            if 0 <= ky < K:
                # [k, c, kx, co]
                wst[:, :, dd, :, s, :] = Wr[:, :, :, ky, :].transpose(2, 1, 3, 0)
    wst = np.ascontiguousarray(wst.reshape(128, 24, 2, COUT)).astype(bf)

    bias = np.ascontiguousarray(np.tile(b, 2).reshape(128, 1))
    zzb = np.zeros((128, 130), bf)
    return btT, sa2, wst, bias, zzb


def _in_maps(template, tomatch, W, b):
    btT, sa2, wst, bias, zzb = _host_prep(template, tomatch, W, b)
    return [
        {"btT": btT[i], "sa2": sa2[i], "wst": wst, "bias": bias, "zzb": zzb}
        for i in range(N_CORES)
    ]


def kernel(template, tomatch, W, b):
    in_maps = _in_maps(template, tomatch, W, b)
    nc = _get_nc()
    res = run_bass_kernel_spmd(nc, in_maps, list(range(N_CORES)))
    return np.stack([res.results[i]["out"] for i in range(N_CORES)])
